# revision 4
# baseline (speedup 1.0000x reference)
"""Trainium2 Bass kernel for nn_PairwiseConv (gnn_message_passing).

Reference computation, for each edge e=(i,j) of a sparse adjacency:
    pair[b,o,e] = sum_c W[o,c,0]*x[b,c,i] + W[o,c,1]*x[b,c,j] + bias[o]
    y[b,o,n]    = (sum_{e: i_e=n} pair[b,o,e]) / max(deg_j[n],1)
    y[b,127,n]  = deg_j[n]            (counts channel)
where deg_j[n] = #{e: j_e = n}.

Algebraic reformulation used here (exact):
    y[b,o,n] = (deg_i[n]*(W0x[b,o,n] + bias[o]) + S[b,o,n]) / max(deg_j[n],1)
    S[b,o,n] = sum_m z[b,o,m] * AT[m,n],   z = W1^T x   (plus an all-ones
               row o=127 so that S[b,127,n] = deg_i[n])
    AT[m,n]  = #{e: j_e = m, i_e = n}  (edge-count matrix)
so the irregular gather/scatter becomes one dense [128,4096]x[4096,512]
matmul per (batch, node-slice) against the on-device-built count matrix.

Sharding: 8 cores = 8 slices of 512 output nodes; each core computes all 4
batches for its slice. AT[:, slice] is built on device from host-packed
per-partition (index,count) tables via GPSIMD local_scatter (32 tiles of
[128 rows, 512 cols], one per 128-row chunk of the source-node axis).
deg_j is built the same way into a [128,512] count matrix C (edges spread
round-robin over the 128 partitions) and reduced with an all-ones matmul,
which also broadcasts deg_j to all 128 partitions.

Host-side work is limited to formatting: slicing/deduplicating edge lists
into padded scatter tables, rotating x so every core sees its slice at
column 0 (keeps the SPMD program identical across cores), and
concatenating the 8 output tiles.
"""

import os

import numpy as np
import ml_dtypes

import concourse.bass as bass
import concourse.mybir as mybir
import concourse.tile as tile
from concourse import bacc
from concourse.bass_utils import run_bass_kernel_spmd

B = 4
C = 128  # in channels
O = 128  # out channels incl. counts row (127 real + ones row)
N = 4096
SLICE = 512  # output nodes per core
NCORES = 8
MC = N // 128  # 32 source-node chunks
F32 = mybir.dt.float32
BF16 = mybir.dt.bfloat16
I16 = mybir.dt.int16
BF16_NP = ml_dtypes.bfloat16


def _pack_tables(rows, cols, nrows, ncols, ni=None):
    """Group (row, col) pairs by partition p=row%128 (and chunk row//128),
    dedup, and pack into [128, nchunk*NI] int16 index / bf16 count tables.

    rows in [0, nrows), cols in [0, ncols). Returns (idx, val, NI).
    """
    nchunk = nrows // 128
    key = rows * ncols + cols
    uniq, counts = np.unique(key, return_counts=True)
    ur = uniq // ncols
    uc = uniq % ncols
    chunk = ur // 128
    p = ur % 128
    # sort by (chunk, p) to get per-(chunk,p) runs
    order = np.lexsort((uc, p, chunk))
    chunk, p, uc, counts = chunk[order], p[order], uc[order], counts[order]
    gid = chunk * 128 + p
    # per (chunk,p) counts
    percell = np.bincount(gid, minlength=nchunk * 128)
    ni = ni if ni is not None else int(percell.max())
    ni += ni % 2  # even
    ni = max(ni, 2)
    idx = np.full((nchunk * 128, ni), -1, np.int16)
    val = np.zeros((nchunk * 128, ni), BF16_NP)
    pos = np.arange(len(gid)) - np.concatenate(([0], np.cumsum(percell)))[gid]
    idx[gid, pos] = uc.astype(np.int16)
    val[gid, pos] = counts.astype(BF16_NP)
    # [nchunk*128, ni] -> [128, nchunk*ni]
    idx = idx.reshape(nchunk, 128, ni).transpose(1, 0, 2).reshape(128, nchunk * ni)
    val = val.reshape(nchunk, 128, ni).transpose(1, 0, 2).reshape(128, nchunk * ni)
    return np.ascontiguousarray(idx), np.ascontiguousarray(val), ni


def prep_inputs(x, W, b, idx_i, idx_j):
    """Returns (in_maps, NI_A, NI_C): per-core input dicts + table widths."""
    x = np.ascontiguousarray(np.asarray(x, np.float32))
    W = np.asarray(W, np.float32)
    bias = np.asarray(b, np.float32)
    ii = np.asarray(idx_i).astype(np.int64)
    jj = np.asarray(idx_j).astype(np.int64)

    # weights: lhsT layouts [K=c, M=o], padded to 128 with a zero column
    W0T = np.zeros((128, 128), np.float32)
    W0T[:, :127] = W[:, :, 0].T
    W1T = np.zeros((128, 128), np.float32)
    W1T[:, :127] = W[:, :, 1].T
    bcol = np.zeros((128, 1), np.float32)
    bcol[:127, 0] = bias

    # first pass: compute per-core tables, track global max widths
    perc = []
    for s in range(NCORES):
        base = s * SLICE
        # AT build: edges with destination i in slice; row = rotated source
        sel = (ii >= base) & (ii < base + SLICE)
        m_rot = (jj[sel] - base) % N
        icol = ii[sel] - base
        a_rows, a_cols = m_rot, icol
        # deg_j build: edges with j in slice, spread over partitions
        selj = (jj >= base) & (jj < base + SLICE)
        nj = int(selj.sum())
        c_rows = np.arange(nj, dtype=np.int64) % 128
        c_cols = jj[selj] - base
        perc.append((a_rows, a_cols, c_rows, c_cols))

    # uniform NI across cores (SPMD program shapes must match)
    ni_a = ni_c = 0
    packed = []
    for a_rows, a_cols, c_rows, c_cols in perc:
        _, _, na = _pack_tables(a_rows, a_cols, N, SLICE)
        _, _, nc_ = _pack_tables(c_rows, c_cols, 128, SLICE)
        ni_a, ni_c = max(ni_a, na), max(ni_c, nc_)

    in_maps = []
    for s in range(NCORES):
        a_rows, a_cols, c_rows, c_cols = perc[s]
        idxA, valA, _ = _pack_tables(a_rows, a_cols, N, SLICE, ni=ni_a)
        idxC, valC, _ = _pack_tables(c_rows, c_cols, 128, SLICE, ni=ni_c)
        m = {
            "W0T": W0T,
            "W1T": W1T,
            "bcol": bcol,
            "idxA": idxA,
            "valA": valA,
            "idxC": idxC,
            "valC": valC,
        }
        for bi in range(B):
            m[f"x{bi}"] = np.ascontiguousarray(np.roll(x[bi], -s * SLICE, axis=1))
        in_maps.append(m)
    return in_maps, ni_a, ni_c


def build_program(ni_a, ni_c):
    nc = bacc.Bacc("TRN2", target_bir_lowering=False, debug=False, num_devices=NCORES)

    xs = [nc.dram_tensor(f"x{bi}", [C, N], F32, kind="ExternalInput") for bi in range(B)]
    W0T = nc.dram_tensor("W0T", [128, 128], F32, kind="ExternalInput")
    W1T = nc.dram_tensor("W1T", [128, 128], F32, kind="ExternalInput")
    bcol = nc.dram_tensor("bcol", [128, 1], F32, kind="ExternalInput")
    idxA = nc.dram_tensor("idxA", [128, MC * ni_a], I16, kind="ExternalInput")
    valA = nc.dram_tensor("valA", [128, MC * ni_a], BF16, kind="ExternalInput")
    idxC = nc.dram_tensor("idxC", [128, ni_c], I16, kind="ExternalInput")
    valC = nc.dram_tensor("valC", [128, ni_c], BF16, kind="ExternalInput")
    yout = nc.dram_tensor("y", [O, B * SLICE], F32, kind="ExternalOutput")

    with tile.TileContext(nc) as tc:
        with (
            tc.tile_pool(name="const", bufs=1) as constp,
            tc.tile_pool(name="scat", bufs=1) as scatp,
            tc.tile_pool(name="at", bufs=1) as atp,
            tc.tile_pool(name="xp", bufs=2) as xp,
            tc.tile_pool(name="zt", bufs=2) as ztp,
            tc.tile_pool(name="work", bufs=1) as workp,
            tc.tile_pool(name="small", bufs=4) as smallp,
            tc.tile_pool(name="ps_zt", bufs=2, space="PSUM") as ps_zt,
            tc.tile_pool(name="ps_s", bufs=2, space="PSUM") as ps_s,
            tc.tile_pool(name="ps_deg", bufs=1, space="PSUM") as ps_deg_p,
            tc.tile_pool(name="ps_u", bufs=1, space="PSUM") as ps_u_p,
            tc.tile_pool(name="ps_di", bufs=1, space="PSUM") as ps_di_p,
        ):
            # ---- constants / tables in ----
            w0t = constp.tile([128, 128], F32)
            nc.sync.dma_start(w0t[:], W0T[:])
            w1t = constp.tile([128, 128], F32)
            nc.sync.dma_start(w1t[:], W1T[:])
            bc = constp.tile([128, 1], F32)
            nc.sync.dma_start(bc[:], bcol[:])
            iA = scatp.tile([128, MC * ni_a], I16)
            nc.sync.dma_start(iA[:], idxA[:])
            vA = scatp.tile([128, MC * ni_a], BF16)
            nc.sync.dma_start(vA[:], valA[:])
            iC = scatp.tile([128, ni_c], I16)
            nc.sync.dma_start(iC[:], idxC[:])
            vC = scatp.tile([128, ni_c], BF16)
            nc.sync.dma_start(vC[:], valC[:])
            ones128 = constp.tile([128, 128], BF16)
            nc.vector.memset(ones128[:], 1.0)
            onescol = constp.tile([1, 128], F32)
            nc.vector.memset(onescol[:], 1.0)

            # ---- deg_j: C count matrix + all-ones matmul reduce ----
            cC = constp.tile([128, SLICE], BF16)
            nc.gpsimd.local_scatter(
                out_ap=cC[:], data_ap=vC[:], idxs_ap=iC[:],
                channels=128, num_elems=SLICE, num_idxs=ni_c,
            )
            ps_deg = ps_deg_p.tile([128, SLICE], F32)
            nc.tensor.matmul(ps_deg[:], ones128[:], cC[:], start=True, stop=True)
            # raw deg row (counts channel) + 1/max(deg,1), broadcast all parts
            degj_raw = smallp.tile([1, SLICE], F32)
            nc.scalar.copy(degj_raw[:], ps_deg[0:1, :])
            rmax = workp.tile([128, SLICE], F32)
            nc.vector.tensor_scalar_max(rmax[:], ps_deg[:], 1.0)
            recip = workp.tile([128, SLICE], F32)
            nc.vector.reciprocal(recip[:], rmax[:])

            # ---- AT: edge-count matrix, one [128,512] tile per source chunk ----
            at = atp.tile([128, MC * SLICE], BF16)
            for mc in range(MC):
                nc.gpsimd.local_scatter(
                    out_ap=at[:, mc * SLICE:(mc + 1) * SLICE],
                    data_ap=vA[:, mc * ni_a:(mc + 1) * ni_a],
                    idxs_ap=iA[:, mc * ni_a:(mc + 1) * ni_a],
                    channels=128, num_elems=SLICE, num_idxs=ni_a,
                )

            # ---- per-batch: zT build, big matmul, epilogue ----
            for bi in range(B):
                xb = xp.tile([C, N], F32, tag="xb")
                half = N // 2
                nc.sync.dma_start(xb[:, :half], xs[bi][:, :half])
                nc.sync.dma_start(xb[:, half:], xs[bi][:, half:])

                # zT[m, o] per chunk: lhsT = x chunk [c, m], rhs = W1T [c, o]
                zt = ztp.tile([128, N], BF16, tag="zt")
                for g in range(MC // 4):  # 8 psum groups of 4 chunks
                    pz = ps_zt.tile([128, 512], F32, tag="pz")
                    for k in range(4):
                        mc = g * 4 + k
                        nc.tensor.matmul(
                            pz[:, k * 128:(k + 1) * 128],
                            xb[:, mc * 128:(mc + 1) * 128],
                            w1t[:],
                            start=True, stop=True,
                        )
                    nc.vector.tensor_copy(zt[:, g * 512:(g + 1) * 512], pz[:])
                # ones row: column o=127 of every chunk
                zt3 = zt[:].rearrange("p (a b) -> p a b", b=128)
                nc.vector.memset(zt3[:, :, 127:128], 1.0)

                # S_b = zT^T-chunks contracted with AT chunks
                ps_S = ps_s.tile([128, SLICE], F32, tag="ps")
                for mc in range(MC):
                    nc.tensor.matmul(
                        ps_S[:],
                        zt[:, mc * 128:(mc + 1) * 128],
                        at[:, mc * SLICE:(mc + 1) * SLICE],
                        start=(mc == 0), stop=(mc == MC - 1),
                        skip_group_check=True,
                    )

                # u_b = W0^T x(slice)  (slice = first 512 cols of rotated x)
                ps_u = ps_u_p.tile([128, SLICE], F32, tag="pu")
                nc.tensor.matmul(ps_u[:], w0t[:], xb[:, :SLICE], start=True, stop=True)
                u_sb = smallp.tile([128, SLICE], F32, tag="usb")
                nc.vector.tensor_scalar_add(u_sb[:], ps_u[:], bc[:, :1])

                # evacuate S; deg_i = its row 127, broadcast to 128 partitions
                s_sb = smallp.tile([128, SLICE], F32, tag="ssb")
                nc.vector.tensor_copy(s_sb[:], ps_S[:])
                degi_row = smallp.tile([1, SLICE], F32, tag="dgr")
                nc.sync.dma_start(degi_row[:], s_sb[127:128, :])
                ps_di = ps_di_p.tile([128, SLICE], F32, tag="pdi")
                nc.tensor.matmul(ps_di[:], onescol[:], degi_row[:], start=True, stop=True)

                # y_b = (S + u*deg_i) * recip ; row 127 = raw deg_j
                t1 = smallp.tile([128, SLICE], F32, tag="t1")
                nc.vector.tensor_mul(t1[:], u_sb[:], ps_di[:])
                t2 = smallp.tile([128, SLICE], F32, tag="t2")
                nc.vector.tensor_add(t2[:], t1[:], s_sb[:])
                ost = workp.tile([O, SLICE], F32, tag="ost")
                nc.vector.tensor_mul(ost[:], t2[:], recip[:])
                nc.sync.dma_start(yout[0:127, bi * SLICE:(bi + 1) * SLICE], ost[0:127, :])
                nc.sync.dma_start(yout[127:128, bi * SLICE:(bi + 1) * SLICE], degj_raw[:])

    nc.compile()
    return nc


def kernel(x, W, b, idx_i, idx_j):
    in_maps, ni_a, ni_c = prep_inputs(x, W, b, idx_i, idx_j)
    nc = build_program(ni_a, ni_c)
    res = run_bass_kernel_spmd(nc, in_maps, list(range(NCORES)))
    y = np.empty((B, O, N), np.float32)
    for s in range(NCORES):
        ys = res.results[s]["y"]  # [O, B*SLICE]
        for bi in range(B):
            y[bi, :, s * SLICE:(s + 1) * SLICE] = ys[:, bi * SLICE:(bi + 1) * SLICE]
    return y


if __name__ == "__main__":
    rng = np.random.default_rng(0)
    x = rng.standard_normal((B, C, N), np.float32)
    W = rng.standard_normal((127, C, 2), np.float32) * 0.05
    b = rng.standard_normal((127,), np.float32) * 0.05
    idx_i = rng.integers(0, N, 131072)
    idx_j = rng.integers(0, N, 131072)
    y = kernel(x, W, b, idx_i, idx_j)
    print("ok", y.shape, float(np.abs(y).mean()))
